# revision 8
# baseline (speedup 1.0000x reference)
"""Deformable conv block (B=8, C=64, H=W=128, K=3) on 8 Trainium2 cores.

Strategy: each SAMPLE is split into 8 y-bands of 16 rows, one band per
NeuronCore, launched as one 8-core SPMD NEFF per sample (8 launches).
Per-core work is 1/8 of a sample, so per-launch HW exec time is ~1/8 of
the single-core kernel.

Device pipeline per core (band rows [y0, y1), bh=16, input rows
[y0-2, y1+2) zero-padded at global image edges by the host):
  1. offset conv: 5x5 conv packed as 5 ky-matmuls with 96 output cols
     (5 kx-taps x 18 chans + bias via ones-row); kx-recombination done
     with 4 full-width shifted DVE adds (NO accumulate-DMAs - those
     wedge multi-core execution); Hardtanh clamp.
  2. mask/weight maps gg (bilinear corner weights x boundary masks) on
     the vector engine in x-partition layout.
  3. sampling: per tap, 9 masked window products + adds on DVE in
     [x; (c, y)] layout from 5 x-shifted transposed copies of x.
  4. finalize: per tap, rearrange + DMA-transpose the sampled band to
     [c; (y, x)] and matmul against W_t with PSUM accumulation ACROSS
     taps; single PSUM->SBUF->DRAM store at the end (f32).

Self-contained: hardcodes all shapes; host packs weights and slices
bands.
"""
import numpy as np
import ml_dtypes
from contextlib import ExitStack

import concourse.bass as bass
from concourse import bacc
import concourse.tile as tile
from concourse import mybir
from concourse.bass_utils import run_bass_kernel_spmd

bf16 = mybir.dt.bfloat16
f32 = mybir.dt.float32
Alu = mybir.AluOpType


def mkap(base_ap, extra_off, free_dims):
    """AP over base_ap's tensor: keep its partition dim, custom free dims."""
    p = list(base_ap.ap[0])
    return bass.AP(base_ap.tensor, base_ap.offset + extra_off, [p] + free_dims)


B, C, H, W = 8, 64, 128, 128
NB = 8           # bands per sample == cores
BH = H // NB     # 16 output rows per band
RIN = BH + 4     # 20 input rows per band (+-2 halo)
FIN = RIN * W    # 2560 flat input positions
FOUT = BH * W    # 2048 flat output positions
NT = 9           # taps
XPF = C * RIN    # xtc tile free size (c-major, y inner)


def _build():
    nc = bacc.Bacc()
    xb = nc.dram_tensor("xb", [C, FIN], bf16, kind="ExternalInput")
    wAll = nc.dram_tensor("wAll", [5, 65, 96], bf16, kind="ExternalInput")
    wM = nc.dram_tensor("wM", [NT, 64, 64], bf16, kind="ExternalInput")
    em = nc.dram_tensor("em", [128, 96], bf16, kind="ExternalInput")
    out = nc.dram_tensor("out", [C, FOUT], f32, kind="ExternalOutput")

    with tile.TileContext(nc, pool_alloc_mode="queue") as tc, ExitStack() as ctx:
        pw = ctx.enter_context(tc.tile_pool(name="pw", bufs=1))
        pxp = ctx.enter_context(tc.tile_pool(name="pxp", bufs=1))
        pxtc = ctx.enter_context(tc.tile_pool(name="pxtc", bufs=1))

        # ---- load x band into padded tile [65, 4 + FIN + 4] ----
        XPW = FIN + 8
        xpad = pxp.tile([65, XPW], bf16, name="xpad")
        nc.vector.memset(xpad[0:64, 0:4], 0.0)
        nc.vector.memset(xpad[0:64, 4 + FIN : XPW], 0.0)
        nc.vector.memset(xpad[64:65, :], 1.0)
        nc.gpsimd.dma_start(xpad[0:64, 4 : 4 + FIN], xb[:])

        # ---- weights ----
        wAt = pw.tile([65, 5 * 96], bf16, name="wAt")
        nc.gpsimd.dma_start(
            wAt[:].rearrange("c (k o) -> c k o", k=5),
            wAll[:].rearrange("k c o -> c k o"),
        )
        wMt = pw.tile([64, NT * 64], bf16, name="wMt")
        nc.gpsimd.dma_start(
            wMt[:].rearrange("c (t o) -> c t o", t=NT),
            wM[:].rearrange("t c o -> c t o"),
        )
        emt = pw.tile([128, 96], bf16, name="emt")
        nc.gpsimd.dma_start(emt[:], em[:])
        offt = pw.tile([128, BH * 32], bf16, name="offt")

        # ---- 5 shifted transposed copies of x ----
        xtc = []
        for si in range(5):
            t_x = pxtc.tile([128, XPF], bf16, name=f"xtc{si}")
            xtc.append(t_x)
        with tc.tile_pool(name="pxty", bufs=2) as pxty:
            for si in range(5):
                s = si - 2
                xty = pxty.tile([128, RIN, 64], bf16, tag="xty", name="xty")
                nc.scalar.dma_start_transpose(
                    xty[:], xpad[0:64, 4 + s : 4 + s + FIN]
                )
                dst = mkap(xtc[si][:], 0, [[RIN, 64], [1, RIN]])
                nc.scalar.copy(out=dst, in_=xty[:].rearrange("x y c -> x c y"))

        # ---- offset conv ----
        # per-kx strips on partitions 0:18 with kx along the FREE dim, so the
        # kx-recombination adds stay partition-aligned (DVE requires it)
        with tc.tile_pool(name="poff", bufs=2) as poff, tc.tile_pool(
            name="psoff", bufs=1, space="PSUM"
        ) as psoff:
            offc = poff.tile([18, 5 * FOUT], bf16, name="offc", bufs=1)
            for q in range(4):
                pA = psoff.tile([18, 5 * 512], f32, tag="pA", name="pA")
                for kx in range(5):
                    for ky in range(5):
                        rhs = xpad[:, 4 + 256 + q * 512 + (ky - 2) * 128 :][:, 0:512]
                        nc.tensor.matmul(
                            pA[:, kx * 512 : (kx + 1) * 512],
                            wAt[:, ky * 96 + kx * 18 : ky * 96 + kx * 18 + 18],
                            rhs,
                            start=(ky == 0), stop=(ky == 4),
                        )
                for kx in range(5):
                    nc.scalar.copy(
                        offc[:, kx * FOUT + q * 512 : kx * FOUT + (q + 1) * 512],
                        pA[:, kx * 512 : (kx + 1) * 512],
                    )

            # kx recombination: off[d,y,x] = sum_kx strip_kx[d, y, x+kx-2]
            offacc = poff.tile([32, FOUT], bf16, name="offacc", bufs=1)
            nc.vector.memset(offacc[:], 0.0)
            nc.scalar.copy(offacc[0:18, :], offc[:, 2 * FOUT : 3 * FOUT])  # kx=2
            oav = offacc[:].rearrange("d (y x) -> d y x", x=W)
            ocv = offc[:].rearrange("d (kx y x) -> d kx y x", kx=5, x=W)
            for kx in [0, 1, 3, 4]:
                co = kx - 2
                xs, xe = max(0, -co), min(W, W - co)
                nc.vector.tensor_tensor(
                    out=oav[0:18, :, xs:xe],
                    in0=oav[0:18, :, xs:xe],
                    in1=ocv[:, kx, :, xs + co : xe + co],
                    op=Alu.add,
                )
            nc.vector.tensor_scalar(
                out=offacc[0:18, :], in0=offacc[0:18, :],
                scalar1=1.0, scalar2=-1.0, op0=Alu.min, op1=Alu.max,
            )
            nc.scalar.dma_start_transpose(
                offt[:].rearrange("x (y d) -> x y d", d=32), offacc[:]
            )

        # ---- mask / weight maps ----
        FB = BH * 32          # 512: one r-block of (y, d)
        pgg = ctx.enter_context(tc.tile_pool(name="pgg", bufs=1))
        gg = pgg.tile([128, 81 * BH], bf16, name="gg")
        with tc.tile_pool(name="pg", bufs=1) as pg:
            mneg = pg.tile([128, FB], bf16, name="mneg")
            nc.vector.tensor_scalar(
                out=mneg[:], in0=offt[:], scalar1=0.0, scalar2=None, op0=Alu.is_lt
            )
            fr = pg.tile([128, FB], bf16, name="fr")
            nc.vector.tensor_tensor(out=fr[:], in0=offt[:], in1=mneg[:], op=Alu.add)
            omf = pg.tile([128, FB], bf16, name="omf")
            nc.vector.tensor_scalar(
                out=omf[:], in0=fr[:], scalar1=-1.0, scalar2=1.0,
                op0=Alu.mult, op1=Alu.add,
            )
            g = pg.tile([128, 3 * FB], bf16, name="g")
            t1 = pg.tile([128, FB], bf16, name="t1")
            g0 = g[:, 0:FB]
            g1 = g[:, FB : 2 * FB]
            g2_ = g[:, 2 * FB : 3 * FB]
            nc.vector.tensor_tensor(out=g0, in0=mneg[:], in1=omf[:], op=Alu.mult)
            nc.vector.tensor_tensor(out=t1[:], in0=mneg[:], in1=fr[:], op=Alu.mult)
            nc.vector.tensor_tensor(out=g2_, in0=fr[:], in1=t1[:], op=Alu.subtract)
            nc.vector.tensor_tensor(out=g1, in0=t1[:], in1=omf[:], op=Alu.add)
            nc.vector.tensor_tensor(out=g1, in0=g1, in1=g0, op=Alu.subtract)

            # x-bound masks: full-width multiply; em is 1.0 except gx cols
            gv = g[:].rearrange("x (r y d) -> x r y d", r=3, d=32)
            for rx in range(3):
                blk = gv[:, rx, :, :]
                em_ap = mkap(emt[:], rx * 32, [[0, BH], [1, 32]])
                nc.vector.tensor_tensor(out=blk, in0=blk, in1=em_ap, op=Alu.mult)

            # gg[x, (ti tj ry rx y)] = gy * gx
            ggv = gg[:].rearrange(
                "x (ti tj ry rx y) -> x ti tj ry rx y", ti=3, tj=3, ry=3, rx=3
            )
            for ry in range(3):
                for ti in range(3):
                    gy_ap = mkap(
                        g[:], ry * FB + 6 * ti,
                        [[2, 3], [0, 3], [32, BH]],
                    )
                    gx_ap = mkap(
                        g[:], 6 * ti + 1,
                        [[2, 3], [FB, 3], [32, BH]],
                    )
                    nc.vector.tensor_tensor(
                        out=ggv[:, ti, :, ry, :, :], in0=gy_ap, in1=gx_ap,
                        op=Alu.mult,
                    )

        # ---- sampling + PSUM-accumulated finalize ----
        pacc = ctx.enter_context(tc.tile_pool(name="pacc", bufs=2))
        ptmp = ctx.enter_context(tc.tile_pool(name="ptmp", bufs=1))
        pfin = ctx.enter_context(tc.tile_pool(name="pfin", bufs=2))
        psm_pool = ctx.enter_context(tc.tile_pool(name="psm", bufs=1, space="PSUM"))

        psm = psm_pool.tile([64, FOUT], f32, name="psm")

        for t in range(NT):
            ti, tj = t // 3, t % 3
            acc = pacc.tile([128, C * BH], bf16, tag="acc", name="acc")
            first = True
            for ry in range(3):
                ro = ti - 2 + ry
                for rx in range(3):
                    co = tj - 2 + rx
                    m = t * 9 + ry * 3 + rx
                    xs_t = xtc[co + 2][:]
                    # read rows (y + 2 + ro) of the 20-row band, c-major
                    in0 = mkap(xs_t, 2 + ro, [[RIN, 64], [1, BH]])
                    in1 = mkap(gg[:], m * BH, [[0, 64], [1, BH]])
                    if first:
                        o_ap = mkap(acc[:], 0, [[BH, 64], [1, BH]])
                        nc.vector.tensor_tensor(
                            out=o_ap, in0=in0, in1=in1, op=Alu.mult
                        )
                        first = False
                    else:
                        tmp = ptmp.tile([128, C * BH], bf16, tag="tmp", name="tmp")
                        nc.vector.tensor_tensor(
                            out=mkap(tmp[:], 0, [[BH, 64], [1, BH]]),
                            in0=in0, in1=in1, op=Alu.mult,
                        )
                        nc.vector.tensor_tensor(
                            out=acc[:], in0=acc[:], in1=tmp[:], op=Alu.add
                        )

            # rearrange acc [x; (c,y)] -> ayc [x; (y, c pad128)], transpose,
            # then matmul with W_t accumulating over taps in PSUM
            ayc = pfin.tile([128, BH * 128], bf16, tag="ayc", name="ayc")
            nc.scalar.copy(
                out=mkap(ayc[:], 0, [[128, BH], [1, 64]]),
                in_=mkap(acc[:], 0, [[1, BH], [BH, 64]]),
            )
            nc.vector.memset(
                mkap(ayc[:], 64, [[128, BH], [1, 64]]), 0.0
            )
            sch = pfin.tile([64, BH, 128], bf16, tag="sch", name="sch")
            nc.sync.dma_start_transpose(sch[:], ayc[:])
            for j in range(4):
                nc.tensor.matmul(
                    psm[:, j * 512 : (j + 1) * 512],
                    wMt[:, t * 64 : (t + 1) * 64],
                    sch[:].rearrange("c a x -> c (a x)")[:, j * 512 : (j + 1) * 512],
                    start=(t == 0), stop=(t == NT - 1),
                )

        outs = pfin.tile([64, FOUT], f32, tag="outs", name="outs", bufs=1)
        nc.scalar.copy(outs[:], psm[:])
        nc.gpsimd.dma_start(out[:], outs[:])

    nc.compile()
    return nc


_NC = None


def _get_nc():
    global _NC
    if _NC is None:
        _NC = _build()
    return _NC


def pack_weights(weights, offset_w, offset_b):
    weights = np.asarray(weights, dtype=np.float32)
    offset_w = np.asarray(offset_w, dtype=np.float32)
    offset_b = np.asarray(offset_b, dtype=np.float32)

    wAll = np.zeros((5, 65, 96), np.float32)
    for kx in range(5):
        # wAll[ky, c, kx*18+d] = offset_w[d, c, ky, kx]
        wAll[:, 0:64, kx * 18 : kx * 18 + 18] = offset_w[:, :, :, kx].transpose(2, 1, 0)
    wAll[2, 64, 36 : 36 + 18] = offset_b
    wM = weights.reshape(C, C, 9).transpose(2, 1, 0).copy()
    em = np.ones((128, 96), np.float32)
    xs_ = np.arange(128)
    for rx in range(3):
        for ti in range(3):
            for tj in range(3):
                co = tj - 2 + rx
                em[:, rx * 32 + 2 * (3 * ti + tj) + 1] = (
                    (xs_ + co >= 0) & (xs_ + co < 128)
                )
    return wAll, wM, em


def pack_inputs(x, weights, offset_w, offset_b):
    """Per-sample list of 8 per-core input maps (one band per core)."""
    x = np.asarray(x, dtype=np.float32)
    wAll, wM, em = pack_weights(weights, offset_w, offset_b)
    cast = lambda a: np.ascontiguousarray(a).astype(ml_dtypes.bfloat16)
    wAll_c, wM_c, em_c = cast(wAll), cast(wM), cast(em)

    # zero-pad rows at global edges once per sample
    sample_maps = []
    for b in range(B):
        xp = np.zeros((C, H + 4, W), np.float32)
        xp[:, 2 : 2 + H, :] = x[b]
        maps = []
        for k in range(NB):
            band = xp[:, k * BH : k * BH + RIN, :]  # rows k*BH-2 .. k*BH+BH+2
            maps.append({
                "xb": cast(band.reshape(C, FIN)),
                "wAll": wAll_c,
                "wM": wM_c,
                "em": em_c,
            })
        sample_maps.append(maps)
    return sample_maps


def kernel(x, weights, offset_w, offset_b):
    x = np.asarray(x, dtype=np.float32)
    sample_maps = pack_inputs(x, weights, offset_w, offset_b)
    nc = _get_nc()
    outs = []
    for b in range(B):
        r = run_bass_kernel_spmd(nc, sample_maps[b], list(range(NB)))
        bands = [np.asarray(r.results[k]["out"]).reshape(C, BH, W) for k in range(NB)]
        outs.append(np.concatenate(bands, axis=1))
    return np.stack(outs).astype(np.float32)


# revision 13
# speedup vs baseline: 8.0853x; 8.0853x over previous
"""Deformable conv block (B=8, C=64, H=W=128, K=3) on 8 Trainium2 cores.

Strategy: each SAMPLE is split into 8 y-bands of 16 rows, one band per
NeuronCore, launched as one 8-core SPMD NEFF per sample (8 launches).
Per-core work is 1/8 of a sample, so per-launch HW exec time is ~1/8 of
the single-core kernel.

Device pipeline per core (band rows [y0, y1), bh=16, input rows
[y0-2, y1+2) zero-padded at global image edges by the host):
  1. offset conv: 5x5 conv packed as 5 ky-matmuls with 96 output cols
     (5 kx-taps x 18 chans + bias via ones-row); kx-recombination done
     with 4 full-width shifted DVE adds (NO accumulate-DMAs - those
     wedge multi-core execution); Hardtanh clamp.
  2. mask/weight maps gg (bilinear corner weights x boundary masks) on
     the vector engine in x-partition layout.
  3. sampling: per tap, 9 masked window products + adds on DVE in
     [x; (c, y)] layout from 5 x-shifted transposed copies of x.
  4. finalize: per tap, rearrange + DMA-transpose the sampled band to
     [c; (y, x)] and matmul against W_t with PSUM accumulation ACROSS
     taps; single PSUM->SBUF->DRAM store at the end (f32).

Self-contained: hardcodes all shapes; host packs weights and slices
bands.
"""
import numpy as np
import ml_dtypes
from contextlib import ExitStack

import concourse.bass as bass
from concourse import bacc
import concourse.tile as tile
from concourse import mybir
from concourse.bass_utils import run_bass_kernel_spmd

bf16 = mybir.dt.bfloat16
f32 = mybir.dt.float32
Alu = mybir.AluOpType


def mkap(base_ap, extra_off, free_dims):
    """AP over base_ap's tensor: keep its partition dim, custom free dims."""
    p = list(base_ap.ap[0])
    return bass.AP(base_ap.tensor, base_ap.offset + extra_off, [p] + free_dims)


B, C, H, W = 8, 64, 128, 128
NB = 8           # bands per sample == cores
BH = H // NB     # 16 output rows per band
RIN = BH + 4     # 20 input rows per band (+-2 halo)
FIN = RIN * W    # 2560 flat input positions
FOUT = BH * W    # 2048 flat output positions
NT = 9           # taps
XPF = C * RIN    # xtc tile free size (c-major, y inner)


def _build():
    nc = bacc.Bacc()
    xb = nc.dram_tensor("xb", [C, FIN], bf16, kind="ExternalInput")
    wAll = nc.dram_tensor("wAll", [5, 65, 96], bf16, kind="ExternalInput")
    wM = nc.dram_tensor("wM", [NT, 64, 64], bf16, kind="ExternalInput")
    em = nc.dram_tensor("em", [128, 96], bf16, kind="ExternalInput")
    out = nc.dram_tensor("out", [C, FOUT], f32, kind="ExternalOutput")

    with tile.TileContext(nc, pool_alloc_mode="queue") as tc, ExitStack() as ctx:
        pw = ctx.enter_context(tc.tile_pool(name="pw", bufs=1))
        pxp = ctx.enter_context(tc.tile_pool(name="pxp", bufs=1))
        pxtc = ctx.enter_context(tc.tile_pool(name="pxtc", bufs=1))

        # ---- load x band into padded tile [65, 4 + FIN + 4] ----
        XPW = FIN + 8
        xpad = pxp.tile([65, XPW], bf16, name="xpad")
        nc.vector.memset(xpad[0:64, 0:4], 0.0)
        nc.vector.memset(xpad[0:64, 4 + FIN : XPW], 0.0)
        nc.vector.memset(xpad[64:65, :], 1.0)
        nc.gpsimd.dma_start(xpad[0:64, 4 : 4 + FIN], xb[:])

        # ---- weights ----
        wAt = pw.tile([65, 5 * 96], bf16, name="wAt")
        nc.gpsimd.dma_start(
            wAt[:].rearrange("c (k o) -> c k o", k=5),
            wAll[:].rearrange("k c o -> c k o"),
        )
        wMt = pw.tile([64, NT * 64], bf16, name="wMt")
        nc.gpsimd.dma_start(
            wMt[:].rearrange("c (t o) -> c t o", t=NT),
            wM[:].rearrange("t c o -> c t o"),
        )
        emt = pw.tile([128, 96], bf16, name="emt")
        nc.gpsimd.dma_start(emt[:], em[:])
        offt = pw.tile([128, BH * 32], bf16, name="offt")

        # ---- 5 shifted transposed copies of x ----
        xtc = []
        for si in range(5):
            t_x = pxtc.tile([128, XPF], bf16, name=f"xtc{si}")
            xtc.append(t_x)
        with tc.tile_pool(name="pxty", bufs=2) as pxty:
            for si in range(5):
                s = si - 2
                xty = pxty.tile([128, RIN, 64], bf16, tag="xty", name="xty")
                nc.scalar.dma_start_transpose(
                    xty[:], xpad[0:64, 4 + s : 4 + s + FIN]
                )
                dst = mkap(xtc[si][:], 0, [[RIN, 64], [1, RIN]])
                nc.scalar.copy(out=dst, in_=xty[:].rearrange("x y c -> x c y"))

        # ---- offset conv ----
        # 20 wide matmuls produce all 5 kx-strips stacked on partitions
        # [kx*18, kx*18+18); plain DMAs (multi-core-safe) restage the strips
        # onto partitions 0:18 with kx along the FREE dim so the DVE
        # kx-recombination adds stay partition-aligned.
        with tc.tile_pool(name="poff", bufs=2) as poff, tc.tile_pool(
            name="psoff", bufs=2, space="PSUM"
        ) as psoff:
            offc96 = poff.tile([96, FOUT], bf16, name="offc96", bufs=1)
            for q in range(4):
                pW = psoff.tile([96, 512], f32, tag="pW", name="pW")
                for ky in range(5):
                    rhs = xpad[:, 4 + 256 + q * 512 + (ky - 2) * 128 :][:, 0:512]
                    nc.tensor.matmul(
                        pW[:], wAt[:, ky * 96 : ky * 96 + 96], rhs,
                        start=(ky == 0), stop=(ky == 4),
                    )
                nc.scalar.copy(offc96[:, q * 512 : (q + 1) * 512], pW[:])
            offc = poff.tile([18, 5 * FOUT], bf16, name="offc", bufs=1)
            for kx in range(5):
                nc.gpsimd.dma_start(
                    offc[:, kx * FOUT : (kx + 1) * FOUT],
                    offc96[kx * 18 : kx * 18 + 18, :],
                )

            # kx recombination: off[d,y,x] = sum_kx strip_kx[d, y, x+kx-2]
            offacc = poff.tile([32, FOUT], bf16, name="offacc", bufs=1)
            nc.vector.memset(offacc[:], 0.0)
            nc.scalar.copy(offacc[0:18, :], offc[:, 2 * FOUT : 3 * FOUT])  # kx=2
            oav = offacc[:].rearrange("d (y x) -> d y x", x=W)
            ocv = offc[:].rearrange("d (kx y x) -> d kx y x", kx=5, x=W)
            for kx in [0, 1, 3, 4]:
                co = kx - 2
                xs, xe = max(0, -co), min(W, W - co)
                nc.vector.tensor_tensor(
                    out=oav[0:18, :, xs:xe],
                    in0=oav[0:18, :, xs:xe],
                    in1=ocv[:, kx, :, xs + co : xe + co],
                    op=Alu.add,
                )
            nc.vector.tensor_scalar(
                out=offacc[0:18, :], in0=offacc[0:18, :],
                scalar1=1.0, scalar2=-1.0, op0=Alu.min, op1=Alu.max,
            )
            nc.scalar.dma_start_transpose(
                offt[:].rearrange("x (y d) -> x y d", d=32), offacc[:]
            )

        # ---- mask / weight maps ----
        FB = BH * 32          # 512: one r-block of (y, d)
        pgg = ctx.enter_context(tc.tile_pool(name="pgg", bufs=1))
        gg = pgg.tile([128, 81 * BH], bf16, name="gg")
        with tc.tile_pool(name="pg", bufs=1) as pg:
            mneg = pg.tile([128, FB], bf16, name="mneg")
            nc.vector.tensor_scalar(
                out=mneg[:], in0=offt[:], scalar1=0.0, scalar2=None, op0=Alu.is_lt
            )
            fr = pg.tile([128, FB], bf16, name="fr")
            nc.vector.tensor_tensor(out=fr[:], in0=offt[:], in1=mneg[:], op=Alu.add)
            omf = pg.tile([128, FB], bf16, name="omf")
            nc.vector.tensor_scalar(
                out=omf[:], in0=fr[:], scalar1=-1.0, scalar2=1.0,
                op0=Alu.mult, op1=Alu.add,
            )
            g = pg.tile([128, 3 * FB], bf16, name="g")
            t1 = pg.tile([128, FB], bf16, name="t1")
            g0 = g[:, 0:FB]
            g1 = g[:, FB : 2 * FB]
            g2_ = g[:, 2 * FB : 3 * FB]
            nc.vector.tensor_tensor(out=g0, in0=mneg[:], in1=omf[:], op=Alu.mult)
            nc.vector.tensor_tensor(out=t1[:], in0=mneg[:], in1=fr[:], op=Alu.mult)
            nc.vector.tensor_tensor(out=g2_, in0=fr[:], in1=t1[:], op=Alu.subtract)
            nc.vector.tensor_tensor(out=g1, in0=t1[:], in1=omf[:], op=Alu.add)
            nc.vector.tensor_tensor(out=g1, in0=g1, in1=g0, op=Alu.subtract)

            # x-bound masks: full-width multiply; em is 1.0 except gx cols
            gv = g[:].rearrange("x (r y d) -> x r y d", r=3, d=32)
            for rx in range(3):
                blk = gv[:, rx, :, :]
                em_ap = mkap(emt[:], rx * 32, [[0, BH], [1, 32]])
                nc.vector.tensor_tensor(out=blk, in0=blk, in1=em_ap, op=Alu.mult)

            # gg[x, (ti tj ry rx y)] = gy * gx
            ggv = gg[:].rearrange(
                "x (ti tj ry rx y) -> x ti tj ry rx y", ti=3, tj=3, ry=3, rx=3
            )
            for ry in range(3):
                for ti in range(3):
                    gy_ap = mkap(
                        g[:], ry * FB + 6 * ti,
                        [[2, 3], [0, 3], [32, BH]],
                    )
                    gx_ap = mkap(
                        g[:], 6 * ti + 1,
                        [[2, 3], [FB, 3], [32, BH]],
                    )
                    nc.vector.tensor_tensor(
                        out=ggv[:, ti, :, ry, :, :], in0=gy_ap, in1=gx_ap,
                        op=Alu.mult,
                    )

        # ---- sampling + PSUM-accumulated finalize ----
        pacc = ctx.enter_context(tc.tile_pool(name="pacc", bufs=2))
        ptmp = ctx.enter_context(tc.tile_pool(name="ptmp", bufs=1))
        pfin = ctx.enter_context(tc.tile_pool(name="pfin", bufs=2))
        psm_pool = ctx.enter_context(tc.tile_pool(name="psm", bufs=1, space="PSUM"))

        psm = psm_pool.tile([64, FOUT], f32, name="psm")

        # pre-zero both rotating ayc buffers; in-loop copies only touch the
        # real-channel half, the pad half must stay zero for the transpose
        for _ in range(2):
            aycp = pfin.tile([128, BH * 128], bf16, tag="ayc", name="aycp")
            nc.gpsimd.memset(aycp[:], 0.0)

        for t in range(NT):
            ti, tj = t // 3, t % 3
            acc = pacc.tile([128, C * BH], bf16, tag="acc", name="acc")
            first = True
            for ry in range(3):
                ro = ti - 2 + ry
                for rx in range(3):
                    co = tj - 2 + rx
                    m = t * 9 + ry * 3 + rx
                    xs_t = xtc[co + 2][:]
                    # read rows (y + 2 + ro) of the 20-row band, c-major
                    in0 = mkap(xs_t, 2 + ro, [[RIN, 64], [1, BH]])
                    in1 = mkap(gg[:], m * BH, [[0, 64], [1, BH]])
                    if first:
                        o_ap = mkap(acc[:], 0, [[BH, 64], [1, BH]])
                        nc.vector.tensor_tensor(
                            out=o_ap, in0=in0, in1=in1, op=Alu.mult
                        )
                        first = False
                    else:
                        tmp = ptmp.tile([128, C * BH], bf16, tag="tmp", name="tmp")
                        nc.vector.tensor_tensor(
                            out=mkap(tmp[:], 0, [[BH, 64], [1, BH]]),
                            in0=in0, in1=in1, op=Alu.mult,
                        )
                        nc.vector.tensor_tensor(
                            out=acc[:], in0=acc[:], in1=tmp[:], op=Alu.add
                        )

            # rearrange acc [x; (c,y)] -> ayc [x; (y, c pad128)], transpose,
            # then matmul with W_t accumulating over taps in PSUM
            ayc = pfin.tile([128, BH * 128], bf16, tag="ayc", name="ayc")
            nc.scalar.copy(
                out=mkap(ayc[:], 0, [[128, BH], [1, 64]]),
                in_=mkap(acc[:], 0, [[1, BH], [BH, 64]]),
            )
            sch = pfin.tile([64, BH, 128], bf16, tag="sch", name="sch")
            nc.sync.dma_start_transpose(sch[:], ayc[:])
            for j in range(4):
                nc.tensor.matmul(
                    psm[:, j * 512 : (j + 1) * 512],
                    wMt[:, t * 64 : (t + 1) * 64],
                    sch[:].rearrange("c a x -> c (a x)")[:, j * 512 : (j + 1) * 512],
                    start=(t == 0), stop=(t == NT - 1),
                )

        outs = pfin.tile([64, FOUT], f32, tag="outs", name="outs", bufs=1)
        nc.scalar.copy(outs[:], psm[:])
        nc.gpsimd.dma_start(out[:], outs[:])

    nc.compile()
    return nc


_NC = None


def _get_nc():
    global _NC
    if _NC is None:
        _NC = _build()
    return _NC


def pack_weights(weights, offset_w, offset_b):
    weights = np.asarray(weights, dtype=np.float32)
    offset_w = np.asarray(offset_w, dtype=np.float32)
    offset_b = np.asarray(offset_b, dtype=np.float32)

    wAll = np.zeros((5, 65, 96), np.float32)
    for kx in range(5):
        # wAll[ky, c, kx*18+d] = offset_w[d, c, ky, kx]
        wAll[:, 0:64, kx * 18 : kx * 18 + 18] = offset_w[:, :, :, kx].transpose(2, 1, 0)
    wAll[2, 64, 36 : 36 + 18] = offset_b
    wM = weights.reshape(C, C, 9).transpose(2, 1, 0).copy()
    em = np.ones((128, 96), np.float32)
    xs_ = np.arange(128)
    for rx in range(3):
        for ti in range(3):
            for tj in range(3):
                co = tj - 2 + rx
                em[:, rx * 32 + 2 * (3 * ti + tj) + 1] = (
                    (xs_ + co >= 0) & (xs_ + co < 128)
                )
    return wAll, wM, em


def pack_inputs(x, weights, offset_w, offset_b):
    """Per-sample list of 8 per-core input maps (one band per core)."""
    x = np.asarray(x, dtype=np.float32)
    wAll, wM, em = pack_weights(weights, offset_w, offset_b)
    cast = lambda a: np.ascontiguousarray(a).astype(ml_dtypes.bfloat16)
    wAll_c, wM_c, em_c = cast(wAll), cast(wM), cast(em)

    # zero-pad rows at global edges once per sample
    sample_maps = []
    for b in range(B):
        xp = np.zeros((C, H + 4, W), np.float32)
        xp[:, 2 : 2 + H, :] = x[b]
        maps = []
        for k in range(NB):
            band = xp[:, k * BH : k * BH + RIN, :]  # rows k*BH-2 .. k*BH+BH+2
            maps.append({
                "xb": cast(band.reshape(C, FIN)),
                "wAll": wAll_c,
                "wM": wM_c,
                "em": em_c,
            })
        sample_maps.append(maps)
    return sample_maps


def kernel(x, weights, offset_w, offset_b):
    x = np.asarray(x, dtype=np.float32)
    sample_maps = pack_inputs(x, weights, offset_w, offset_b)
    nc = _get_nc()
    outs = []
    for b in range(B):
        r = run_bass_kernel_spmd(nc, sample_maps[b], list(range(NB)))
        bands = [np.asarray(r.results[k]["out"]).reshape(C, BH, W) for k in range(NB)]
        outs.append(np.concatenate(bands, axis=1))
    return np.stack(outs).astype(np.float32)


# revision 16
# speedup vs baseline: 15.5377x; 1.9217x over previous
"""Deformable conv block (B=8, C=64, H=W=128, K=3) on 8 Trainium2 cores.

Strategy: each SAMPLE is split into 8 y-bands of 16 rows, one band per
NeuronCore, launched as one 8-core SPMD NEFF per sample (8 launches).
Per-core work is 1/8 of a sample, so per-launch HW exec time is ~1/8 of
the single-core kernel.

Device pipeline per core (band rows [y0, y1), bh=16, input rows
[y0-2, y1+2) zero-padded at global image edges by the host):
  1. offset conv: 5x5 conv packed as 5 ky-matmuls with 96 output cols
     (5 kx-taps x 18 chans + bias via ones-row); kx-recombination done
     with 4 full-width shifted DVE adds (NO accumulate-DMAs - those
     wedge multi-core execution); Hardtanh clamp.
  2. mask/weight maps gg (bilinear corner weights x boundary masks) on
     the vector engine in x-partition layout.
  3. sampling: per tap, 9 masked window products + adds on DVE in
     [x; (c, y)] layout from 5 x-shifted transposed copies of x.
  4. finalize: per tap, rearrange + DMA-transpose the sampled band to
     [c; (y, x)] and matmul against W_t with PSUM accumulation ACROSS
     taps; single PSUM->SBUF->DRAM store at the end (f32).

Self-contained: hardcodes all shapes; host packs weights and slices
bands.
"""
import numpy as np
import ml_dtypes
from contextlib import ExitStack

import concourse.bass as bass
from concourse import bacc
import concourse.tile as tile
from concourse import mybir
from concourse.bass_utils import run_bass_kernel_spmd

bf16 = mybir.dt.bfloat16
f32 = mybir.dt.float32
Alu = mybir.AluOpType


def mkap(base_ap, extra_off, free_dims):
    """AP over base_ap's tensor: keep its partition dim, custom free dims."""
    p = list(base_ap.ap[0])
    return bass.AP(base_ap.tensor, base_ap.offset + extra_off, [p] + free_dims)


B, C, H, W = 8, 64, 128, 128
NB = 8           # cores per launch
SPLIT = 32       # bands per sample (SPLIT // NB launches per sample)
BH = H // SPLIT  # output rows per band
RIN = BH + 4     # input rows per band (+-2 halo)
FIN = RIN * W    # flat input positions
FOUT = BH * W    # flat output positions
NQ = FOUT // 512 # 512-wide position blocks
NT = 9           # taps
XPF = C * RIN    # xtc tile free size (c-major, y inner)


def _build():
    nc = bacc.Bacc()
    xb = nc.dram_tensor("xb", [C, FIN], bf16, kind="ExternalInput")
    wAll = nc.dram_tensor("wAll", [5, 65, 96], bf16, kind="ExternalInput")
    wM = nc.dram_tensor("wM", [NT, 64, 64], bf16, kind="ExternalInput")
    em = nc.dram_tensor("em", [128, 96], bf16, kind="ExternalInput")
    out = nc.dram_tensor("out", [C, FOUT], f32, kind="ExternalOutput")

    with tile.TileContext(nc, pool_alloc_mode="queue") as tc, ExitStack() as ctx:
        pw = ctx.enter_context(tc.tile_pool(name="pw", bufs=1))
        pxp = ctx.enter_context(tc.tile_pool(name="pxp", bufs=1))
        pxtc = ctx.enter_context(tc.tile_pool(name="pxtc", bufs=1))

        # ---- load x band into padded tile [65, 4 + FIN + 4] ----
        XPW = FIN + 8
        xpad = pxp.tile([65, XPW], bf16, name="xpad")
        nc.vector.memset(xpad[0:64, 0:4], 0.0)
        nc.vector.memset(xpad[0:64, 4 + FIN : XPW], 0.0)
        nc.vector.memset(xpad[64:65, :], 1.0)
        nc.gpsimd.dma_start(xpad[0:64, 4 : 4 + FIN], xb[:])

        # ---- weights ----
        wAt = pw.tile([65, 5 * 96], bf16, name="wAt")
        nc.gpsimd.dma_start(
            wAt[:].rearrange("c (k o) -> c k o", k=5),
            wAll[:].rearrange("k c o -> c k o"),
        )
        wMt = pw.tile([64, NT * 64], bf16, name="wMt")
        nc.gpsimd.dma_start(
            wMt[:].rearrange("c (t o) -> c t o", t=NT),
            wM[:].rearrange("t c o -> c t o"),
        )
        emt = pw.tile([128, 96], bf16, name="emt")
        nc.gpsimd.dma_start(emt[:], em[:])
        offt = pw.tile([128, BH * 32], bf16, name="offt")

        # ---- 5 shifted transposed copies of x ----
        xtc = []
        for si in range(5):
            t_x = pxtc.tile([128, XPF], bf16, name=f"xtc{si}")
            xtc.append(t_x)
        with tc.tile_pool(name="pxty", bufs=2) as pxty:
            for si in range(5):
                s = si - 2
                xty = pxty.tile([128, RIN, 64], bf16, tag="xty", name="xty")
                nc.scalar.dma_start_transpose(
                    xty[:], xpad[0:64, 4 + s : 4 + s + FIN]
                )
                dst = mkap(xtc[si][:], 0, [[RIN, 64], [1, RIN]])
                nc.scalar.copy(out=dst, in_=xty[:].rearrange("x y c -> x c y"))

        # ---- offset conv ----
        # 20 wide matmuls produce all 5 kx-strips stacked on partitions
        # [kx*18, kx*18+18); plain DMAs (multi-core-safe) restage the strips
        # onto partitions 0:18 with kx along the FREE dim so the DVE
        # kx-recombination adds stay partition-aligned.
        with tc.tile_pool(name="poff", bufs=2) as poff, tc.tile_pool(
            name="psoff", bufs=2, space="PSUM"
        ) as psoff:
            offc96 = poff.tile([96, FOUT], bf16, name="offc96", bufs=1)
            for q in range(NQ):
                pW = psoff.tile([96, 512], f32, tag="pW", name="pW")
                for ky in range(5):
                    rhs = xpad[:, 4 + 256 + q * 512 + (ky - 2) * 128 :][:, 0:512]
                    nc.tensor.matmul(
                        pW[:], wAt[:, ky * 96 : ky * 96 + 96], rhs,
                        start=(ky == 0), stop=(ky == 4),
                    )
                nc.scalar.copy(offc96[:, q * 512 : (q + 1) * 512], pW[:])
            offc = poff.tile([18, 5 * FOUT], bf16, name="offc", bufs=1)
            for kx in range(5):
                nc.gpsimd.dma_start(
                    offc[:, kx * FOUT : (kx + 1) * FOUT],
                    offc96[kx * 18 : kx * 18 + 18, :],
                )

            # kx recombination: off[d,y,x] = sum_kx strip_kx[d, y, x+kx-2]
            offacc = poff.tile([32, FOUT], bf16, name="offacc", bufs=1)
            nc.vector.memset(offacc[:], 0.0)
            nc.scalar.copy(offacc[0:18, :], offc[:, 2 * FOUT : 3 * FOUT])  # kx=2
            oav = offacc[:].rearrange("d (y x) -> d y x", x=W)
            ocv = offc[:].rearrange("d (kx y x) -> d kx y x", kx=5, x=W)
            for kx in [0, 1, 3, 4]:
                co = kx - 2
                xs, xe = max(0, -co), min(W, W - co)
                nc.vector.tensor_tensor(
                    out=oav[0:18, :, xs:xe],
                    in0=oav[0:18, :, xs:xe],
                    in1=ocv[:, kx, :, xs + co : xe + co],
                    op=Alu.add,
                )
            nc.vector.tensor_scalar(
                out=offacc[0:18, :], in0=offacc[0:18, :],
                scalar1=1.0, scalar2=-1.0, op0=Alu.min, op1=Alu.max,
            )
            nc.scalar.dma_start_transpose(
                offt[:].rearrange("x (y d) -> x y d", d=32), offacc[:]
            )

        # ---- mask / weight maps ----
        FB = BH * 32          # 512: one r-block of (y, d)
        pgg = ctx.enter_context(tc.tile_pool(name="pgg", bufs=1))
        gg = pgg.tile([128, 81 * BH], bf16, name="gg")
        with tc.tile_pool(name="pg", bufs=1) as pg:
            mneg = pg.tile([128, FB], bf16, name="mneg")
            nc.vector.tensor_scalar(
                out=mneg[:], in0=offt[:], scalar1=0.0, scalar2=None, op0=Alu.is_lt
            )
            fr = pg.tile([128, FB], bf16, name="fr")
            nc.vector.tensor_tensor(out=fr[:], in0=offt[:], in1=mneg[:], op=Alu.add)
            omf = pg.tile([128, FB], bf16, name="omf")
            nc.vector.tensor_scalar(
                out=omf[:], in0=fr[:], scalar1=-1.0, scalar2=1.0,
                op0=Alu.mult, op1=Alu.add,
            )
            g = pg.tile([128, 3 * FB], bf16, name="g")
            t1 = pg.tile([128, FB], bf16, name="t1")
            g0 = g[:, 0:FB]
            g1 = g[:, FB : 2 * FB]
            g2_ = g[:, 2 * FB : 3 * FB]
            nc.vector.tensor_tensor(out=g0, in0=mneg[:], in1=omf[:], op=Alu.mult)
            nc.vector.tensor_tensor(out=t1[:], in0=mneg[:], in1=fr[:], op=Alu.mult)
            nc.vector.tensor_tensor(out=g2_, in0=fr[:], in1=t1[:], op=Alu.subtract)
            nc.vector.tensor_tensor(out=g1, in0=t1[:], in1=omf[:], op=Alu.add)
            nc.vector.tensor_tensor(out=g1, in0=g1, in1=g0, op=Alu.subtract)

            # x-bound masks: full-width multiply; em is 1.0 except gx cols
            gv = g[:].rearrange("x (r y d) -> x r y d", r=3, d=32)
            for rx in range(3):
                blk = gv[:, rx, :, :]
                em_ap = mkap(emt[:], rx * 32, [[0, BH], [1, 32]])
                nc.vector.tensor_tensor(out=blk, in0=blk, in1=em_ap, op=Alu.mult)

            # gg[x, (ti tj ry rx y)] = gy * gx
            ggv = gg[:].rearrange(
                "x (ti tj ry rx y) -> x ti tj ry rx y", ti=3, tj=3, ry=3, rx=3
            )
            for ry in range(3):
                for ti in range(3):
                    gy_ap = mkap(
                        g[:], ry * FB + 6 * ti,
                        [[2, 3], [0, 3], [32, BH]],
                    )
                    gx_ap = mkap(
                        g[:], 6 * ti + 1,
                        [[2, 3], [FB, 3], [32, BH]],
                    )
                    nc.vector.tensor_tensor(
                        out=ggv[:, ti, :, ry, :, :], in0=gy_ap, in1=gx_ap,
                        op=Alu.mult,
                    )

        # ---- sampling + PSUM-accumulated finalize ----
        pacc = ctx.enter_context(tc.tile_pool(name="pacc", bufs=2))
        ptmp = ctx.enter_context(tc.tile_pool(name="ptmp", bufs=1))
        pfin = ctx.enter_context(tc.tile_pool(name="pfin", bufs=2))
        psm_pool = ctx.enter_context(tc.tile_pool(name="psm", bufs=1, space="PSUM"))

        psm = psm_pool.tile([64, FOUT], f32, name="psm")

        # pre-zero both rotating ayc buffers; in-loop copies only touch the
        # real-channel half, the pad half must stay zero for the transpose
        for _ in range(2):
            aycp = pfin.tile([128, BH * 128], bf16, tag="ayc", name="aycp")
            nc.gpsimd.memset(aycp[:], 0.0)

        for t in range(NT):
            ti, tj = t // 3, t % 3
            acc = pacc.tile([128, C * BH], bf16, tag="acc", name="acc")
            first = True
            for ry in range(3):
                ro = ti - 2 + ry
                for rx in range(3):
                    co = tj - 2 + rx
                    m = t * 9 + ry * 3 + rx
                    xs_t = xtc[co + 2][:]
                    # read rows (y + 2 + ro) of the 20-row band, c-major
                    in0 = mkap(xs_t, 2 + ro, [[RIN, 64], [1, BH]])
                    in1 = mkap(gg[:], m * BH, [[0, 64], [1, BH]])
                    if first:
                        o_ap = mkap(acc[:], 0, [[BH, 64], [1, BH]])
                        nc.vector.tensor_tensor(
                            out=o_ap, in0=in0, in1=in1, op=Alu.mult
                        )
                        first = False
                    else:
                        tmp = ptmp.tile([128, C * BH], bf16, tag="tmp", name="tmp")
                        nc.vector.tensor_tensor(
                            out=mkap(tmp[:], 0, [[BH, 64], [1, BH]]),
                            in0=in0, in1=in1, op=Alu.mult,
                        )
                        nc.vector.tensor_tensor(
                            out=acc[:], in0=acc[:], in1=tmp[:], op=Alu.add
                        )

            # rearrange acc [x; (c,y)] -> ayc [x; (y, c pad128)], transpose,
            # then matmul with W_t accumulating over taps in PSUM
            ayc = pfin.tile([128, BH * 128], bf16, tag="ayc", name="ayc")
            nc.scalar.copy(
                out=mkap(ayc[:], 0, [[128, BH], [1, 64]]),
                in_=mkap(acc[:], 0, [[1, BH], [BH, 64]]),
            )
            sch = pfin.tile([64, BH, 128], bf16, tag="sch", name="sch")
            nc.sync.dma_start_transpose(sch[:], ayc[:])
            for j in range(NQ):
                nc.tensor.matmul(
                    psm[:, j * 512 : (j + 1) * 512],
                    wMt[:, t * 64 : (t + 1) * 64],
                    sch[:].rearrange("c a x -> c (a x)")[:, j * 512 : (j + 1) * 512],
                    start=(t == 0), stop=(t == NT - 1),
                )

        outs = pfin.tile([64, FOUT], f32, tag="outs", name="outs", bufs=1)
        nc.scalar.copy(outs[:], psm[:])
        nc.gpsimd.dma_start(out[:], outs[:])

    nc.compile()
    return nc


_NC = None


def _get_nc():
    global _NC
    if _NC is None:
        _NC = _build()
    return _NC


def pack_weights(weights, offset_w, offset_b):
    weights = np.asarray(weights, dtype=np.float32)
    offset_w = np.asarray(offset_w, dtype=np.float32)
    offset_b = np.asarray(offset_b, dtype=np.float32)

    wAll = np.zeros((5, 65, 96), np.float32)
    for kx in range(5):
        # wAll[ky, c, kx*18+d] = offset_w[d, c, ky, kx]
        wAll[:, 0:64, kx * 18 : kx * 18 + 18] = offset_w[:, :, :, kx].transpose(2, 1, 0)
    wAll[2, 64, 36 : 36 + 18] = offset_b
    wM = weights.reshape(C, C, 9).transpose(2, 1, 0).copy()
    em = np.ones((128, 96), np.float32)
    xs_ = np.arange(128)
    for rx in range(3):
        for ti in range(3):
            for tj in range(3):
                co = tj - 2 + rx
                em[:, rx * 32 + 2 * (3 * ti + tj) + 1] = (
                    (xs_ + co >= 0) & (xs_ + co < 128)
                )
    return wAll, wM, em


def pack_inputs(x, weights, offset_w, offset_b):
    """Per-sample list of 8 per-core input maps (one band per core)."""
    x = np.asarray(x, dtype=np.float32)
    wAll, wM, em = pack_weights(weights, offset_w, offset_b)
    cast = lambda a: np.ascontiguousarray(a).astype(ml_dtypes.bfloat16)
    wAll_c, wM_c, em_c = cast(wAll), cast(wM), cast(em)

    # zero-pad rows at global edges once per sample
    sample_maps = []
    for b in range(B):
        xp = np.zeros((C, H + 4, W), np.float32)
        xp[:, 2 : 2 + H, :] = x[b]
        maps = []
        for k in range(SPLIT):
            band = xp[:, k * BH : k * BH + RIN, :]  # rows k*BH-2 .. k*BH+BH+2
            maps.append({
                "xb": cast(band.reshape(C, FIN)),
                "wAll": wAll_c,
                "wM": wM_c,
                "em": em_c,
            })
        sample_maps.append(maps)
    return sample_maps


def kernel(x, weights, offset_w, offset_b):
    x = np.asarray(x, dtype=np.float32)
    sample_maps = pack_inputs(x, weights, offset_w, offset_b)
    nc = _get_nc()
    outs = []
    for b in range(B):
        bands = []
        for g in range(SPLIT // NB):
            r = run_bass_kernel_spmd(nc, sample_maps[b][g * NB : (g + 1) * NB], list(range(NB)))
            bands += [np.asarray(r.results[k]["out"]).reshape(C, BH, W) for k in range(NB)]
        outs.append(np.concatenate(bands, axis=1))
    return np.stack(outs).astype(np.float32)


# revision 18
# speedup vs baseline: 16.3419x; 1.0518x over previous
"""Deformable conv block (B=8, C=64, H=W=128, K=3) on 8 Trainium2 cores.

Strategy: each SAMPLE is split into SPLIT y-bands of H/SPLIT rows; each
8-core SPMD launch processes 8 consecutive bands (SPLIT//8 launches per
sample). Per-core, per-launch work is 1/SPLIT of a sample, so per-launch
HW exec time shrinks accordingly (measured: SPLIT=8 -> 191us,
SPLIT=16 -> 130us, SPLIT=32 -> 100us; fixed per-launch overhead ~70us).

Device pipeline per core (band rows [y0, y1), bh=H/SPLIT, input rows
[y0-2, y1+2) zero-padded at global image edges by the host):
  1. offset conv: 5x5 conv packed as 5 ky-matmuls with 96 output cols
     (5 kx-taps x 18 chans + bias via ones-row); kx-recombination done
     with 4 full-width shifted DVE adds (NO accumulate-DMAs - those
     wedge multi-core execution); Hardtanh clamp.
  2. mask/weight maps gg (bilinear corner weights x boundary masks) on
     the vector engine in x-partition layout.
  3. sampling: per tap, 9 masked window products + adds on DVE in
     [x; (c, y)] layout from 5 x-shifted transposed copies of x.
  4. finalize: per tap, rearrange + DMA-transpose the sampled band to
     [c; (y, x)] and matmul against W_t with PSUM accumulation ACROSS
     taps; single PSUM->SBUF->DRAM store at the end (f32).

Self-contained: hardcodes all shapes; host packs weights and slices
bands.
"""
import numpy as np
import ml_dtypes
from contextlib import ExitStack

import concourse.bass as bass
from concourse import bacc
import concourse.tile as tile
from concourse import mybir
from concourse.bass_utils import run_bass_kernel_spmd

bf16 = mybir.dt.bfloat16
f32 = mybir.dt.float32
Alu = mybir.AluOpType


def mkap(base_ap, extra_off, free_dims):
    """AP over base_ap's tensor: keep its partition dim, custom free dims."""
    p = list(base_ap.ap[0])
    return bass.AP(base_ap.tensor, base_ap.offset + extra_off, [p] + free_dims)


B, C, H, W = 8, 64, 128, 128
NB = 8           # cores per launch
SPLIT = 32       # bands per sample (SPLIT // NB launches per sample)
BH = H // SPLIT  # output rows per band
RIN = BH + 4     # input rows per band (+-2 halo)
FIN = RIN * W    # flat input positions
FOUT = BH * W    # flat output positions
NQ = FOUT // 512 # 512-wide position blocks
NT = 9           # taps
XPF = C * RIN    # xtc tile free size (c-major, y inner)


def _build():
    nc = bacc.Bacc()
    xb = nc.dram_tensor("xb", [C, FIN], bf16, kind="ExternalInput")
    wAll = nc.dram_tensor("wAll", [5, 65, 96], bf16, kind="ExternalInput")
    wM = nc.dram_tensor("wM", [NT, 64, 64], bf16, kind="ExternalInput")
    em = nc.dram_tensor("em", [128, 96], bf16, kind="ExternalInput")
    out = nc.dram_tensor("out", [C, FOUT], f32, kind="ExternalOutput")

    with tile.TileContext(nc, pool_alloc_mode="queue") as tc, ExitStack() as ctx:
        pw = ctx.enter_context(tc.tile_pool(name="pw", bufs=1))
        pxp = ctx.enter_context(tc.tile_pool(name="pxp", bufs=1))
        pxtc = ctx.enter_context(tc.tile_pool(name="pxtc", bufs=1))

        # ---- load x band into padded tile [65, 4 + FIN + 4] ----
        XPW = FIN + 8
        xpad = pxp.tile([65, XPW], bf16, name="xpad")
        nc.vector.memset(xpad[0:64, 0:4], 0.0)
        nc.vector.memset(xpad[0:64, 4 + FIN : XPW], 0.0)
        nc.vector.memset(xpad[64:65, :], 1.0)
        nc.gpsimd.dma_start(xpad[0:64, 4 : 4 + FIN], xb[:])

        # ---- weights ----
        wAt = pw.tile([65, 5 * 96], bf16, name="wAt")
        nc.gpsimd.dma_start(
            wAt[:].rearrange("c (k o) -> c k o", k=5),
            wAll[:].rearrange("k c o -> c k o"),
        )
        wMt = pw.tile([64, NT * 64], bf16, name="wMt")
        nc.gpsimd.dma_start(
            wMt[:].rearrange("c (t o) -> c t o", t=NT),
            wM[:].rearrange("t c o -> c t o"),
        )
        emt = pw.tile([128, 96], bf16, name="emt")
        nc.gpsimd.dma_start(emt[:], em[:])
        offt = pw.tile([128, BH * 32], bf16, name="offt")

        # ---- 5 shifted transposed copies of x ----
        xtc = []
        for si in range(5):
            t_x = pxtc.tile([128, XPF], bf16, name=f"xtc{si}")
            xtc.append(t_x)
        with tc.tile_pool(name="pxty", bufs=2) as pxty:
            for si in range(5):
                s = si - 2
                xty = pxty.tile([128, RIN, 64], bf16, tag="xty", name="xty")
                nc.sync.dma_start_transpose(
                    xty[:], xpad[0:64, 4 + s : 4 + s + FIN]
                )
                dst = mkap(xtc[si][:], 0, [[RIN, 64], [1, RIN]])
                nc.vector.tensor_copy(out=dst, in_=xty[:].rearrange("x y c -> x c y"))

        # ---- offset conv ----
        # 20 wide matmuls produce all 5 kx-strips stacked on partitions
        # [kx*18, kx*18+18); plain DMAs (multi-core-safe) restage the strips
        # onto partitions 0:18 with kx along the FREE dim so the DVE
        # kx-recombination adds stay partition-aligned.
        with tc.tile_pool(name="poff", bufs=2) as poff, tc.tile_pool(
            name="psoff", bufs=2, space="PSUM"
        ) as psoff:
            offc96 = poff.tile([96, FOUT], bf16, name="offc96", bufs=1)
            for q in range(NQ):
                pW = psoff.tile([96, 512], f32, tag="pW", name="pW")
                for ky in range(5):
                    rhs = xpad[:, 4 + 256 + q * 512 + (ky - 2) * 128 :][:, 0:512]
                    nc.tensor.matmul(
                        pW[:], wAt[:, ky * 96 : ky * 96 + 96], rhs,
                        start=(ky == 0), stop=(ky == 4),
                    )
                nc.scalar.copy(offc96[:, q * 512 : (q + 1) * 512], pW[:])
            offc = poff.tile([18, 5 * FOUT], bf16, name="offc", bufs=1)
            for kx in range(5):
                nc.gpsimd.dma_start(
                    offc[:, kx * FOUT : (kx + 1) * FOUT],
                    offc96[kx * 18 : kx * 18 + 18, :],
                )

            # kx recombination: off[d,y,x] = sum_kx strip_kx[d, y, x+kx-2]
            offacc = poff.tile([32, FOUT], bf16, name="offacc", bufs=1)
            nc.vector.memset(offacc[:], 0.0)
            nc.scalar.copy(offacc[0:18, :], offc[:, 2 * FOUT : 3 * FOUT])  # kx=2
            oav = offacc[:].rearrange("d (y x) -> d y x", x=W)
            ocv = offc[:].rearrange("d (kx y x) -> d kx y x", kx=5, x=W)
            for kx in [0, 1, 3, 4]:
                co = kx - 2
                xs, xe = max(0, -co), min(W, W - co)
                nc.vector.tensor_tensor(
                    out=oav[0:18, :, xs:xe],
                    in0=oav[0:18, :, xs:xe],
                    in1=ocv[:, kx, :, xs + co : xe + co],
                    op=Alu.add,
                )
            nc.vector.tensor_scalar(
                out=offacc[0:18, :], in0=offacc[0:18, :],
                scalar1=1.0, scalar2=-1.0, op0=Alu.min, op1=Alu.max,
            )
            nc.scalar.dma_start_transpose(
                offt[:].rearrange("x (y d) -> x y d", d=32), offacc[:]
            )

        # ---- mask / weight maps ----
        FB = BH * 32          # 512: one r-block of (y, d)
        pgg = ctx.enter_context(tc.tile_pool(name="pgg", bufs=1))
        gg = pgg.tile([128, 81 * BH], bf16, name="gg")
        with tc.tile_pool(name="pg", bufs=1) as pg:
            mneg = pg.tile([128, FB], bf16, name="mneg")
            nc.vector.tensor_scalar(
                out=mneg[:], in0=offt[:], scalar1=0.0, scalar2=None, op0=Alu.is_lt
            )
            fr = pg.tile([128, FB], bf16, name="fr")
            nc.vector.tensor_tensor(out=fr[:], in0=offt[:], in1=mneg[:], op=Alu.add)
            omf = pg.tile([128, FB], bf16, name="omf")
            nc.vector.tensor_scalar(
                out=omf[:], in0=fr[:], scalar1=-1.0, scalar2=1.0,
                op0=Alu.mult, op1=Alu.add,
            )
            g = pg.tile([128, 3 * FB], bf16, name="g")
            t1 = pg.tile([128, FB], bf16, name="t1")
            g0 = g[:, 0:FB]
            g1 = g[:, FB : 2 * FB]
            g2_ = g[:, 2 * FB : 3 * FB]
            nc.vector.tensor_tensor(out=g0, in0=mneg[:], in1=omf[:], op=Alu.mult)
            nc.vector.tensor_tensor(out=t1[:], in0=mneg[:], in1=fr[:], op=Alu.mult)
            nc.vector.tensor_tensor(out=g2_, in0=fr[:], in1=t1[:], op=Alu.subtract)
            nc.vector.tensor_tensor(out=g1, in0=t1[:], in1=omf[:], op=Alu.add)
            nc.vector.tensor_tensor(out=g1, in0=g1, in1=g0, op=Alu.subtract)

            # x-bound masks: full-width multiply; em is 1.0 except gx cols
            gv = g[:].rearrange("x (r y d) -> x r y d", r=3, d=32)
            for rx in range(3):
                blk = gv[:, rx, :, :]
                em_ap = mkap(emt[:], rx * 32, [[0, BH], [1, 32]])
                nc.vector.tensor_tensor(out=blk, in0=blk, in1=em_ap, op=Alu.mult)

            # gg[x, (ti tj ry rx y)] = gy * gx
            ggv = gg[:].rearrange(
                "x (ti tj ry rx y) -> x ti tj ry rx y", ti=3, tj=3, ry=3, rx=3
            )
            for ti in range(3):
                for ry in range(3):
                    gy_ap = mkap(
                        g[:], ry * FB + 6 * ti,
                        [[2, 3], [0, 3], [32, BH]],
                    )
                    gx_ap = mkap(
                        g[:], 6 * ti + 1,
                        [[2, 3], [FB, 3], [32, BH]],
                    )
                    nc.vector.tensor_tensor(
                        out=ggv[:, ti, :, ry, :, :], in0=gy_ap, in1=gx_ap,
                        op=Alu.mult,
                    )

        # ---- sampling + PSUM-accumulated finalize ----
        pacc = ctx.enter_context(tc.tile_pool(name="pacc", bufs=2))
        ptmp = ctx.enter_context(tc.tile_pool(name="ptmp", bufs=1))
        pfin = ctx.enter_context(tc.tile_pool(name="pfin", bufs=2))
        psm_pool = ctx.enter_context(tc.tile_pool(name="psm", bufs=1, space="PSUM"))

        psm = psm_pool.tile([64, FOUT], f32, name="psm")

        # pre-zero both rotating ayc buffers; in-loop copies only touch the
        # real-channel half, the pad half must stay zero for the transpose
        for _ in range(2):
            aycp = pfin.tile([128, BH * 128], bf16, tag="ayc", name="aycp")
            nc.gpsimd.memset(aycp[:], 0.0)

        for t in range(NT):
            ti, tj = t // 3, t % 3
            acc = pacc.tile([128, C * BH], bf16, tag="acc", name="acc")
            first = True
            for ry in range(3):
                ro = ti - 2 + ry
                for rx in range(3):
                    co = tj - 2 + rx
                    m = t * 9 + ry * 3 + rx
                    xs_t = xtc[co + 2][:]
                    # read rows (y + 2 + ro) of the 20-row band, c-major
                    in0 = mkap(xs_t, 2 + ro, [[RIN, 64], [1, BH]])
                    in1 = mkap(gg[:], m * BH, [[0, 64], [1, BH]])
                    if first:
                        o_ap = mkap(acc[:], 0, [[BH, 64], [1, BH]])
                        nc.vector.tensor_tensor(
                            out=o_ap, in0=in0, in1=in1, op=Alu.mult
                        )
                        first = False
                    else:
                        tmp = ptmp.tile([128, C * BH], bf16, tag="tmp", name="tmp")
                        nc.vector.tensor_tensor(
                            out=mkap(tmp[:], 0, [[BH, 64], [1, BH]]),
                            in0=in0, in1=in1, op=Alu.mult,
                        )
                        nc.vector.tensor_tensor(
                            out=acc[:], in0=acc[:], in1=tmp[:], op=Alu.add
                        )

            # rearrange acc [x; (c,y)] -> ayc [x; (y, c pad128)], transpose,
            # then matmul with W_t accumulating over taps in PSUM
            ayc = pfin.tile([128, BH * 128], bf16, tag="ayc", name="ayc")
            nc.scalar.copy(
                out=mkap(ayc[:], 0, [[128, BH], [1, 64]]),
                in_=mkap(acc[:], 0, [[1, BH], [BH, 64]]),
            )
            sch = pfin.tile([64, BH, 128], bf16, tag="sch", name="sch")
            nc.sync.dma_start_transpose(sch[:], ayc[:])
            for j in range(NQ):
                nc.tensor.matmul(
                    psm[:, j * 512 : (j + 1) * 512],
                    wMt[:, t * 64 : (t + 1) * 64],
                    sch[:].rearrange("c a x -> c (a x)")[:, j * 512 : (j + 1) * 512],
                    start=(t == 0), stop=(t == NT - 1),
                )

        outs = pfin.tile([64, FOUT], f32, tag="outs", name="outs", bufs=1)
        nc.scalar.copy(outs[:], psm[:])
        nc.gpsimd.dma_start(out[:], outs[:])

    nc.compile()
    return nc


_NC = None


def _get_nc():
    global _NC
    if _NC is None:
        _NC = _build()
    return _NC


def pack_weights(weights, offset_w, offset_b):
    weights = np.asarray(weights, dtype=np.float32)
    offset_w = np.asarray(offset_w, dtype=np.float32)
    offset_b = np.asarray(offset_b, dtype=np.float32)

    wAll = np.zeros((5, 65, 96), np.float32)
    for kx in range(5):
        # wAll[ky, c, kx*18+d] = offset_w[d, c, ky, kx]
        wAll[:, 0:64, kx * 18 : kx * 18 + 18] = offset_w[:, :, :, kx].transpose(2, 1, 0)
    wAll[2, 64, 36 : 36 + 18] = offset_b
    wM = weights.reshape(C, C, 9).transpose(2, 1, 0).copy()
    em = np.ones((128, 96), np.float32)
    xs_ = np.arange(128)
    for rx in range(3):
        for ti in range(3):
            for tj in range(3):
                co = tj - 2 + rx
                em[:, rx * 32 + 2 * (3 * ti + tj) + 1] = (
                    (xs_ + co >= 0) & (xs_ + co < 128)
                )
    return wAll, wM, em


def pack_inputs(x, weights, offset_w, offset_b):
    """Per-sample list of 8 per-core input maps (one band per core)."""
    x = np.asarray(x, dtype=np.float32)
    wAll, wM, em = pack_weights(weights, offset_w, offset_b)
    cast = lambda a: np.ascontiguousarray(a).astype(ml_dtypes.bfloat16)
    wAll_c, wM_c, em_c = cast(wAll), cast(wM), cast(em)

    # zero-pad rows at global edges once per sample
    sample_maps = []
    for b in range(B):
        xp = np.zeros((C, H + 4, W), np.float32)
        xp[:, 2 : 2 + H, :] = x[b]
        maps = []
        for k in range(SPLIT):
            band = xp[:, k * BH : k * BH + RIN, :]  # rows k*BH-2 .. k*BH+BH+2
            maps.append({
                "xb": cast(band.reshape(C, FIN)),
                "wAll": wAll_c,
                "wM": wM_c,
                "em": em_c,
            })
        sample_maps.append(maps)
    return sample_maps


def kernel(x, weights, offset_w, offset_b):
    x = np.asarray(x, dtype=np.float32)
    sample_maps = pack_inputs(x, weights, offset_w, offset_b)
    nc = _get_nc()
    outs = []
    for b in range(B):
        bands = []
        for g in range(SPLIT // NB):
            r = run_bass_kernel_spmd(nc, sample_maps[b][g * NB : (g + 1) * NB], list(range(NB)))
            bands += [np.asarray(r.results[k]["out"]).reshape(C, BH, W) for k in range(NB)]
        outs.append(np.concatenate(bands, axis=1))
    return np.stack(outs).astype(np.float32)
